# revision 13
# baseline (speedup 1.0000x reference)
"""Trainium2 Bass kernel for the CRN (GRU-based) model.

Strategy (pure data parallel, batch sharded 8 ways, 128 examples/core):

Per core, batch Bc=128 sits exactly on the 128 SBUF partitions ("A-space"
layout: [batch, feature]).  Per timestep the recurrent work is expressed as
three accumulating fp32r matmuls into two PSUM banks:

  psumE [128, 512] = [ r | z | y1pre | gxn ]
      = I @ C_E            (per-example constants: x_enc projections + biases)
      + catvar_T.T @ Rvar  (time-varying embedding/y features, K=49)
      + h_T.T @ Wh_E       (recurrent part, cols 0:384)
  psumF [128, 256] = [ ghn | t1pre ]
      = I @ C_F            (bias broadcasts)
      + h_T.T @ Wh_F

Gates/activations run on ACT (sigmoid/tanh/erf all live in the same ACT
table set -> no table reloads; exact gelu(x) = 0.5*x*(1+erf(x/sqrt(2)))
with the 0.5 folded into the host-side output projection).  The GRU combine
runs on DVE.  h' is transposed back to [hid, batch] via PE transpose + ACT
copy to feed the next step's matmuls.

The tiny output heads (128->4 and 128->1) are deferred to the host: the
kernel stores 2*gelu(pre) activations in fp16 and the host applies the
[256 -> 5] projection with numpy.  h0 and all per-example constants are
precomputed on the host (exact f32), matching the reference bit-for-bit on
those terms.
"""

import sys

sys.path.insert(0, "/opt/trn_rl_repo")

import numpy as np

import concourse.bass as bass
import concourse.bacc as bacc
import concourse.mybir as mybir
import concourse.tile as tile
from concourse.tile import add_dep_helper
from concourse.bass_utils import run_bass_kernel_spmd

B, S, DX, DH = 1024, 512, 128, 128
NCORES = 8
BC = B // NCORES  # 128
KVAR = 49  # ae(32) + te(16) + y(1)
SG = 4  # steps per DMA group
NG = S // SG

F32 = mybir.dt.float32
F32R = mybir.dt.float32r
F16 = mybir.dt.float16
AFT = mybir.ActivationFunctionType
ALU = mybir.AluOpType

INV_SQRT2 = 0.7071067811865476


def _build_program():
    nc = bacc.Bacc("TRN2", target_bir_lowering=False, debug=False)

    # --- DRAM parameters (per-core values supplied via in_maps) ---
    d_h0T = nc.declare_dram_parameter("h0T", [128, BC], F16, isOutput=False)
    d_h0 = nc.declare_dram_parameter("h0", [BC, 128], F16, isOutput=False)
    d_cat = nc.declare_dram_parameter("catT4", [NG, KVAR, SG * BC], F16, isOutput=False)
    d_CE = nc.declare_dram_parameter("CE", [BC, 512], F16, isOutput=False)
    d_CF = nc.declare_dram_parameter("CF", [BC, 256], F16, isOutput=False)
    d_WhE = nc.declare_dram_parameter("WhE", [128, 384], F16, isOutput=False)
    d_WhF = nc.declare_dram_parameter("WhF", [128, 256], F16, isOutput=False)
    d_Rv = nc.declare_dram_parameter("Rvar", [KVAR, 512], F16, isOutput=False)
    d_id = nc.declare_dram_parameter("ident", [128, 128], F16, isOutput=False)
    d_OH = nc.declare_dram_parameter("OUTH", [BC, S, 128], F16, isOutput=True)
    d_O2 = nc.declare_dram_parameter("OUT2", [BC, S, 256], F16, isOutput=True)

    def r32(ap):
        return ap.bitcast(F32R)

    with tile.TileContext(nc) as tc:
        with (
            tc.tile_pool(name="const", bufs=1) as cpool,
            tc.tile_pool(name="state", bufs=3) as spool,
            tc.tile_pool(name="work", bufs=3) as wpool,
            tc.tile_pool(name="io", bufs=4) as iopool,
            tc.tile_pool(name="psE", bufs=3, space="PSUM") as pE,
            tc.tile_pool(name="psF", bufs=3, space="PSUM") as pF,
            tc.tile_pool(name="psT", bufs=2, space="PSUM") as pT,
        ):
            def const_r(name, dram):
                t = cpool.tile(list(dram.shape), dram.dtype, name=name)
                nc.sync.dma_start(out=t[:], in_=dram.ap())
                return t

            sCE = const_r("sCE", d_CE)
            sCF = const_r("sCF", d_CF)
            sWhE = const_r("sWhE", d_WhE)
            sWhF = const_r("sWhF", d_WhF)
            sRv = const_r("sRv", d_Rv)
            sId = const_r("sId", d_id)

            hT = const_r("hT0", d_h0T)
            hS = cpool.tile([BC, 128], F16, name="hS0")
            nc.sync.dma_start(out=hS[:], in_=d_h0.ap())

            # rolling stage tiles for the h output: stage[g][:, j, :] = h(4g+j)
            stage_h = {}

            def get_stage(g):
                if g not in stage_h:
                    t = iopool.tile([BC, SG, 128], F16, tag="stH", name=f"stH{g}")
                    stage_h[g] = t
                return stage_h[g]

            # h(0) slot is filled by the host; memset so the DMA reads
            # initialized memory.
            nc.vector.memset(get_stage(0)[:, 0, :], 0.0)

            for g in range(NG):
                cat4 = iopool.tile([KVAR, SG * BC], F16, tag="cat", name=f"cat{g}")
                nc.sync.dma_start(out=cat4[:], in_=d_cat.ap()[g])
                stH = get_stage(g)
                st2 = iopool.tile([BC, SG, 256], F16, tag="st2", name=f"st2{g}")

                for j in range(SG):
                    s = SG * g + j
                    catv = cat4[:, BC * j : BC * (j + 1)]

                    psumE = pE.tile([128, 512], F32, tag="E", name=f"psE{s}")
                    psumF = pF.tile([128, 256], F32, tag="F", name=f"psF{s}")

                    mm = nc.tensor.matmul
                    # constants
                    mm(psumE[:, 0:512], sId[:], sCE[:],
                       start=True, stop=False, skip_group_check=True)
                    mm(psumF[:, 0:256], sId[:], sCF[:],
                       start=True, stop=False, skip_group_check=True)
                    # time-varying features
                    mm(psumE[:, 0:512], catv, sRv[:],
                       start=False, stop=False, skip_group_check=True)
                    # recurrent part: r/z first (the recurrence chain
                    # needs them), then ghn/t1, then y1 (heads only)
                    mm(psumE[:, 0:256], hT[:], sWhE[:, 0:256],
                       start=False, stop=True, skip_group_check=True)
                    mm(psumF[:, 0:256], hT[:], sWhF[:],
                       start=False, stop=True, skip_group_check=True)
                    mm(psumE[:, 384:512], hT[:], sWhE[:, 256:384],
                       start=False, stop=True, skip_group_check=True)

                    act = nc.scalar.activation
                    stt = nc.vector.scalar_tensor_tensor

                    if s < S - 1:
                        # ---- recurrence chain (queued first on ACT/DVE) ----
                        rS = wpool.tile([BC, 128], F32, tag="r", name=f"r{s}")
                        zS = wpool.tile([BC, 128], F32, tag="z", name=f"z{s}")
                        zcS = wpool.tile([BC, 128], F32, tag="zc", name=f"zc{s}")
                        act(out=rS[:], in_=psumE[:, 0:128], func=AFT.Sigmoid)
                        act(out=zS[:], in_=psumE[:, 128:256], func=AFT.Sigmoid)
                        act(out=zcS[:], in_=psumE[:, 128:256], func=AFT.Sigmoid,
                            scale=-1.0)

                        tmp1 = wpool.tile([BC, 128], F32, tag="tmp1", name=f"t1_{s}")
                        pren = wpool.tile([BC, 128], F32, tag="pren", name=f"pn{s}")
                        nS = wpool.tile([BC, 128], F32, tag="n", name=f"n{s}")
                        zhS = wpool.tile([BC, 128], F32, tag="zh", name=f"zh{s}")
                        nzc = wpool.tile([BC, 128], F32, tag="nzc", name=f"nz{s}")

                        i_mul = nc.vector.tensor_mul(tmp1[:], psumF[:, 0:128], rS[:])
                        i_add = nc.vector.tensor_add(pren[:], tmp1[:], psumE[:, 256:384])
                        act(out=nS[:], in_=pren[:], func=AFT.Tanh)
                        i_zh = nc.vector.tensor_mul(zhS[:], zS[:], hS[:])
                        add_dep_helper(i_zh.ins, i_add.ins, sync=False,
                                       reason="keep chain add ahead of zh on DVE")
                        nc.vector.tensor_mul(nzc[:], nS[:], zcS[:])

                        s2 = s + 1
                        hp = get_stage(s2 // SG)[:, s2 % SG, :]
                        nc.vector.tensor_add(hp, nzc[:], zhS[:])
                        hS = hp

                        psumT = pT.tile([128, BC], F16, tag="T", name=f"psT{s}")
                        nc.tensor.transpose(psumT[:], hp, sId[:])
                        hT = spool.tile([128, BC], F16, tag="hT", name=f"hT{s}")
                        nc.vector.tensor_copy(hT[:], psumT[:])

                    # ---- output heads (off the chain, queued last) ----
                    eS = wpool.tile([BC, 256], F32, tag="e", name=f"e{s}")
                    act(out=eS[:, 0:128], in_=psumF[:, 128:256], func=AFT.Erf,
                        scale=INV_SQRT2)
                    act(out=eS[:, 128:256], in_=psumE[:, 384:512], func=AFT.Erf,
                        scale=INV_SQRT2)
                    stt(out=st2[:, j, 0:128], in0=eS[:, 0:128], scalar=1.0,
                        in1=psumF[:, 128:256], op0=ALU.add, op1=ALU.mult)
                    stt(out=st2[:, j, 128:256], in0=eS[:, 128:256], scalar=1.0,
                        in1=psumE[:, 384:512], op0=ALU.add, op1=ALU.mult)

                # group writeback
                nc.sync.dma_start(out=d_OH.ap()[:, SG * g : SG * (g + 1), :],
                                  in_=stH[:])
                nc.sync.dma_start(out=d_O2.ap()[:, SG * g : SG * (g + 1), :],
                                  in_=st2[:])
                del stage_h[g]

    nc.compile()
    return nc


def kernel(x, a, t, y, mask, xW, xb, a_emb, t_emb, W_ih, b_ih, W_hh, b_hh,
           h0W, h0b, tW1, tb1, tW2, tb2, yW1, yb1, yW2, yb2):
    f = np.float32
    x = np.asarray(x, f)
    y = np.asarray(y, f)
    a = np.asarray(a)
    t = np.asarray(t)
    xW, xb = np.asarray(xW, f), np.asarray(xb, f)
    a_emb, t_emb = np.asarray(a_emb, f), np.asarray(t_emb, f)
    W_ih, b_ih = np.asarray(W_ih, f), np.asarray(b_ih, f)
    W_hh, b_hh = np.asarray(W_hh, f), np.asarray(b_hh, f)
    h0W, h0b = np.asarray(h0W, f), np.asarray(h0b, f)
    tW1, tb1 = np.asarray(tW1, f), np.asarray(tb1, f)
    tW2, tb2 = np.asarray(tW2, f), np.asarray(tb2, f)
    yW1, yb1 = np.asarray(yW1, f), np.asarray(yb1, f)
    yW2, yb2 = np.asarray(yW2, f), np.asarray(yb2, f)

    # ---- host precompute (exact f32) ----
    x_enc = (x @ xW.T + xb).astype(f)
    h0 = np.tanh(x_enc @ h0W.T + h0b).astype(f)

    CE = np.concatenate(
        [
            x_enc @ W_ih[0:128, 0:128].T + b_ih[0:128] + b_hh[0:128],
            x_enc @ W_ih[128:256, 0:128].T + b_ih[128:256] + b_hh[128:256],
            x_enc @ W_ih[256:384, 0:128].T + b_ih[256:384],
            x_enc @ yW1[:, 128:256].T + yb1,
        ],
        axis=1,
    ).astype(f)  # [B, 512]
    CF = np.ascontiguousarray(
        np.broadcast_to(np.concatenate([b_hh[256:384], tb1]).astype(f), (BC, 256))
    )
    WhE = np.ascontiguousarray(
        np.hstack([W_hh[0:128].T, W_hh[128:256].T, yW1[:, 0:128].T]).astype(f)
    )
    WhF = np.ascontiguousarray(np.hstack([W_hh[256:384].T, tW1.T]).astype(f))

    Rvar = np.zeros((KVAR, 512), f)
    Rvar[:, 0:128] = W_ih[0:128, 128:177].T
    Rvar[:, 128:256] = W_ih[128:256, 128:177].T
    Rvar[:, 256:384] = W_ih[256:384, 128:177].T
    Rvar[0:48, 384:512] = yW1[:, 256:304].T
    ident = np.eye(128, dtype=f)

    # catvar [B, S, 49] = [a_emb[a] | t_emb[t] | y]
    catvar = np.empty((B, S, KVAR), f)
    catvar[:, :, 0:32] = a_emb[a]
    catvar[:, :, 32:48] = t_emb[t]
    catvar[:, :, 48] = y

    nc = _build_program()

    in_maps = []
    for c in range(NCORES):
        sl = slice(c * BC, (c + 1) * BC)
        cv = catvar[sl]  # [128, S, 49]
        # -> [NG, KVAR, SG*BC]: catT4[g, k, j*BC + b] = cv[b, 4g+j, k]
        cT = np.ascontiguousarray(cv.transpose(1, 2, 0))  # [S, 49, 128]
        cT4 = np.ascontiguousarray(
            cT.reshape(NG, SG, KVAR, BC).transpose(0, 2, 1, 3)
        ).reshape(NG, KVAR, SG * BC)
        h16 = np.float16
        in_maps.append(
            {
                "h0T": np.ascontiguousarray(h0[sl].T).astype(h16),
                "h0": np.ascontiguousarray(h0[sl]).astype(h16),
                "catT4": cT4.astype(h16),
                "CE": np.ascontiguousarray(CE[sl]).astype(h16),
                "CF": CF.astype(h16),
                "WhE": WhE.astype(h16),
                "WhF": WhF.astype(h16),
                "Rvar": Rvar.astype(h16),
                "ident": ident.astype(h16),
            }
        )

    res = run_bass_kernel_spmd(nc, in_maps, list(range(NCORES)))

    y_seq = np.empty((B, S, 1), f)
    t_seq = np.empty((B, S, 4), f)
    h_seq = np.empty((B, S, 128), f)
    for c in range(NCORES):
        sl = slice(c * BC, (c + 1) * BC)
        OH = res.results[c]["OUTH"].astype(f)  # [128, S, 128]
        O2 = res.results[c]["OUT2"].astype(f)  # [128, S, 256]
        h_seq[sl] = OH
        h_seq[sl, 0, :] = h0[sl]
        # ty = 2*gelu(pre); heads: logits = gelu(pre) @ W.T + b
        t_seq[sl] = 0.5 * (O2[:, :, 0:128] @ tW2.T) + tb2
        y_seq[sl] = 0.5 * (O2[:, :, 128:256] @ yW2.T) + yb2

    return (y_seq, t_seq, h_seq)


# revision 14
# speedup vs baseline: 1.1448x; 1.1448x over previous
"""Trainium2 Bass kernel for the CRN (GRU-based) model.

Strategy (pure data parallel, batch sharded 8 ways, 128 examples/core):

Per core, batch Bc=128 sits exactly on the 128 SBUF partitions ("A-space"
layout: [batch, feature]).  Per timestep the recurrent work is expressed as
three accumulating fp32r matmuls into two PSUM banks:

  psumE [128, 512] = [ r | z | y1pre | gxn ]
      = I @ C_E            (per-example constants: x_enc projections + biases)
      + catvar_T.T @ Rvar  (time-varying embedding/y features, K=49)
      + h_T.T @ Wh_E       (recurrent part, cols 0:384)
  psumF [128, 256] = [ ghn | t1pre ]
      = I @ C_F            (bias broadcasts)
      + h_T.T @ Wh_F

Gates/activations run on ACT (sigmoid/tanh/erf all live in the same ACT
table set -> no table reloads; exact gelu(x) = 0.5*x*(1+erf(x/sqrt(2)))
with the 0.5 folded into the host-side output projection).  The GRU combine
runs on DVE.  h' is transposed back to [hid, batch] via PE transpose + ACT
copy to feed the next step's matmuls.

The tiny output heads (128->4 and 128->1) are deferred to the host: the
kernel stores 2*gelu(pre) activations in fp16 and the host applies the
[256 -> 5] projection with numpy.  h0 and all per-example constants are
precomputed on the host (exact f32), matching the reference bit-for-bit on
those terms.
"""

import sys

sys.path.insert(0, "/opt/trn_rl_repo")

import numpy as np

import concourse.bass as bass
import concourse.bacc as bacc
import concourse.mybir as mybir
import concourse.tile as tile
from concourse.tile import add_dep_helper
from concourse.bass_utils import run_bass_kernel_spmd

B, S, DX, DH = 1024, 512, 128, 128
NCORES = 8
BC = B // NCORES  # 128
KVAR = 49  # ae(32) + te(16) + y(1)
SG = 4  # steps per DMA group
NG = S // SG

F32 = mybir.dt.float32
F32R = mybir.dt.float32r
F16 = mybir.dt.float16
AFT = mybir.ActivationFunctionType
ALU = mybir.AluOpType

INV_SQRT2 = 0.7071067811865476


def _build_program():
    nc = bacc.Bacc("TRN2", target_bir_lowering=False, debug=False)

    # --- DRAM parameters (per-core values supplied via in_maps) ---
    d_h0T = nc.declare_dram_parameter("h0T", [128, BC], F16, isOutput=False)
    d_h0 = nc.declare_dram_parameter("h0", [BC, 128], F16, isOutput=False)
    d_cat = nc.declare_dram_parameter("catT4", [NG, KVAR, SG * BC], F16, isOutput=False)
    d_CE = nc.declare_dram_parameter("CE", [BC, 512], F16, isOutput=False)
    d_CF = nc.declare_dram_parameter("CF", [BC, 256], F16, isOutput=False)
    d_WhE = nc.declare_dram_parameter("WhE", [128, 384], F16, isOutput=False)
    d_WhF = nc.declare_dram_parameter("WhF", [128, 256], F16, isOutput=False)
    d_Rv = nc.declare_dram_parameter("Rvar", [KVAR, 512], F16, isOutput=False)
    d_id = nc.declare_dram_parameter("ident", [128, 128], F16, isOutput=False)
    d_OH = nc.declare_dram_parameter("OUTH", [BC, S, 128], F16, isOutput=True)
    d_O2 = nc.declare_dram_parameter("OUT2", [BC, S, 256], F16, isOutput=True)

    def r32(ap):
        return ap.bitcast(F32R)

    with tile.TileContext(nc) as tc:
        with (
            tc.tile_pool(name="const", bufs=1) as cpool,
            tc.tile_pool(name="state", bufs=3) as spool,
            tc.tile_pool(name="work", bufs=3) as wpool,
            tc.tile_pool(name="io", bufs=4) as iopool,
            tc.tile_pool(name="psE", bufs=3, space="PSUM") as pE,
            tc.tile_pool(name="psF", bufs=3, space="PSUM") as pF,
            tc.tile_pool(name="psT", bufs=2, space="PSUM") as pT,
        ):
            def const_r(name, dram):
                t = cpool.tile(list(dram.shape), dram.dtype, name=name)
                nc.sync.dma_start(out=t[:], in_=dram.ap())
                return t

            sCE = const_r("sCE", d_CE)
            sCF = const_r("sCF", d_CF)
            sWhE = const_r("sWhE", d_WhE)
            sWhF = const_r("sWhF", d_WhF)
            sRv = const_r("sRv", d_Rv)
            sId = const_r("sId", d_id)

            hT = const_r("hT0", d_h0T)
            hS = cpool.tile([BC, 128], F16, name="hS0")
            nc.sync.dma_start(out=hS[:], in_=d_h0.ap())

            # rolling stage tiles for the h output: stage[g][:, j, :] = h(4g+j)
            stage_h = {}

            def get_stage(g):
                if g not in stage_h:
                    t = iopool.tile([BC, SG, 128], F16, tag="stH", name=f"stH{g}")
                    stage_h[g] = t
                return stage_h[g]

            # h(0) slot is filled by the host; memset so the DMA reads
            # initialized memory.
            nc.vector.memset(get_stage(0)[:, 0, :], 0.0)

            act = nc.scalar.activation
            stt = nc.vector.scalar_tensor_tensor
            mm = nc.tensor.matmul

            # HAM warm-up: ~16 back-to-back 512-col matmuls give the PE a
            # fully-busy activity window, releasing the 1.2GHz clock gate.
            # Steady-state gaps stay < the ~3.4us idle window, so the PE
            # never re-throttles.
            psumW = pE.tile([128, 512], F32, tag="E", name="psWarm")
            for k in range(16):
                mm(psumW[:, 0:512], sId[:], sCE[:],
                   start=True, stop=True, skip_group_check=True)

            st2_tiles = {}
            cat_tiles = {}
            # heads deferred one step: (s, psumE, psumF)
            pending = None

            def emit_heads(p, i_tanh, i_copy):
                ps, pE_, pF_ = p
                pg, pj = divmod(ps, SG)
                st2 = st2_tiles[pg]
                eS = wpool.tile([BC, 256], F32, tag="e", name=f"e{ps}")
                i_e1 = act(out=eS[:, 0:128], in_=pF_[:, 128:256], func=AFT.Erf,
                           scale=INV_SQRT2)
                i_e2 = act(out=eS[:, 128:256], in_=pE_[:, 384:512], func=AFT.Erf,
                           scale=INV_SQRT2)
                i_s1 = stt(out=st2[:, pj, 0:128], in0=eS[:, 0:128], scalar=1.0,
                           in1=pF_[:, 128:256], op0=ALU.add, op1=ALU.mult)
                i_s2 = stt(out=st2[:, pj, 128:256], in0=eS[:, 128:256], scalar=1.0,
                           in1=pE_[:, 384:512], op0=ALU.add, op1=ALU.mult)
                if i_tanh is not None:
                    add_dep_helper(i_e1.ins, i_tanh.ins, sync=False,
                                   reason="heads after chain on ACT")
                if i_copy is not None:
                    add_dep_helper(i_s1.ins, i_copy.ins, sync=False,
                                   reason="heads after chain on DVE")
                if pj == SG - 1:
                    nc.sync.dma_start(
                        out=d_O2.ap()[:, SG * pg : SG * (pg + 1), :], in_=st2[:])
                    del st2_tiles[pg]

            for s in range(S):
                g, j = divmod(s, SG)
                if j == 0:
                    cat4 = iopool.tile([KVAR, SG * BC], F16, tag="cat",
                                       name=f"cat{g}")
                    nc.sync.dma_start(out=cat4[:], in_=d_cat.ap()[g])
                    cat_tiles[g] = cat4
                    st2_tiles[g] = iopool.tile([BC, SG, 256], F16, tag="st2",
                                               name=f"st2{g}")
                catv = cat_tiles[g][:, BC * j : BC * (j + 1)]

                psumE = pE.tile([128, 512], F32, tag="E", name=f"psE{s}")
                psumF = pF.tile([128, 256], F32, tag="F", name=f"psF{s}")

                # constants
                mm(psumE[:, 0:512], sId[:], sCE[:],
                   start=True, stop=False, skip_group_check=True)
                mm(psumF[:, 0:256], sId[:], sCF[:],
                   start=True, stop=False, skip_group_check=True)
                # time-varying features
                mm(psumE[:, 0:512], catv, sRv[:],
                   start=False, stop=False, skip_group_check=True)
                # recurrent part: r/z first (the chain needs them first)
                mm(psumE[:, 0:256], hT[:], sWhE[:, 0:256],
                   start=False, stop=True, skip_group_check=True)
                mm(psumF[:, 0:256], hT[:], sWhF[:],
                   start=False, stop=True, skip_group_check=True)
                mm(psumE[:, 384:512], hT[:], sWhE[:, 256:384],
                   start=False, stop=True, skip_group_check=True)

                i_tanh = i_copy = None
                if s < S - 1:
                    # ---- recurrence chain ----
                    rS = wpool.tile([BC, 128], F32, tag="r", name=f"r{s}")
                    zS = wpool.tile([BC, 128], F32, tag="z", name=f"z{s}")
                    zcS = wpool.tile([BC, 128], F32, tag="zc", name=f"zc{s}")
                    act(out=rS[:], in_=psumE[:, 0:128], func=AFT.Sigmoid)
                    act(out=zS[:], in_=psumE[:, 128:256], func=AFT.Sigmoid)
                    act(out=zcS[:], in_=psumE[:, 128:256], func=AFT.Sigmoid,
                        scale=-1.0)

                    tmp1 = wpool.tile([BC, 128], F32, tag="tmp1", name=f"t1_{s}")
                    pren = wpool.tile([BC, 128], F32, tag="pren", name=f"pn{s}")
                    nS = wpool.tile([BC, 128], F32, tag="n", name=f"n{s}")
                    zhS = wpool.tile([BC, 128], F32, tag="zh", name=f"zh{s}")
                    nzc = wpool.tile([BC, 128], F32, tag="nzc", name=f"nz{s}")

                    i_mul = nc.vector.tensor_mul(tmp1[:], psumF[:, 0:128], rS[:])
                    i_add = nc.vector.tensor_add(pren[:], tmp1[:],
                                                 psumE[:, 256:384])
                    i_tanh = act(out=nS[:], in_=pren[:], func=AFT.Tanh)
                    i_zh = nc.vector.tensor_mul(zhS[:], zS[:], hS[:])
                    add_dep_helper(i_zh.ins, i_add.ins, sync=False,
                                   reason="keep chain add ahead of zh on DVE")
                    nc.vector.tensor_mul(nzc[:], nS[:], zcS[:])

                    s2 = s + 1
                    hp = get_stage(s2 // SG)[:, s2 % SG, :]
                    nc.vector.tensor_add(hp, nzc[:], zhS[:])
                    hS = hp

                    psumT = pT.tile([128, BC], F16, tag="T", name=f"psT{s}")
                    nc.tensor.transpose(psumT[:], hp, sId[:])
                    hT = spool.tile([128, BC], F16, tag="hT", name=f"hT{s}")
                    i_copy = nc.vector.tensor_copy(hT[:], psumT[:])

                # ---- heads of the PREVIOUS step (fill engine gaps) ----
                if pending is not None:
                    emit_heads(pending, i_tanh, i_copy)
                pending = (s, psumE, psumF)

                if j == SG - 1:
                    nc.sync.dma_start(
                        out=d_OH.ap()[:, SG * g : SG * (g + 1), :],
                        in_=get_stage(g)[:])
                    del stage_h[g]

            emit_heads(pending, None, None)

    nc.compile()
    return nc


def kernel(x, a, t, y, mask, xW, xb, a_emb, t_emb, W_ih, b_ih, W_hh, b_hh,
           h0W, h0b, tW1, tb1, tW2, tb2, yW1, yb1, yW2, yb2):
    f = np.float32
    x = np.asarray(x, f)
    y = np.asarray(y, f)
    a = np.asarray(a)
    t = np.asarray(t)
    xW, xb = np.asarray(xW, f), np.asarray(xb, f)
    a_emb, t_emb = np.asarray(a_emb, f), np.asarray(t_emb, f)
    W_ih, b_ih = np.asarray(W_ih, f), np.asarray(b_ih, f)
    W_hh, b_hh = np.asarray(W_hh, f), np.asarray(b_hh, f)
    h0W, h0b = np.asarray(h0W, f), np.asarray(h0b, f)
    tW1, tb1 = np.asarray(tW1, f), np.asarray(tb1, f)
    tW2, tb2 = np.asarray(tW2, f), np.asarray(tb2, f)
    yW1, yb1 = np.asarray(yW1, f), np.asarray(yb1, f)
    yW2, yb2 = np.asarray(yW2, f), np.asarray(yb2, f)

    # ---- host precompute (exact f32) ----
    x_enc = (x @ xW.T + xb).astype(f)
    h0 = np.tanh(x_enc @ h0W.T + h0b).astype(f)

    CE = np.concatenate(
        [
            x_enc @ W_ih[0:128, 0:128].T + b_ih[0:128] + b_hh[0:128],
            x_enc @ W_ih[128:256, 0:128].T + b_ih[128:256] + b_hh[128:256],
            x_enc @ W_ih[256:384, 0:128].T + b_ih[256:384],
            x_enc @ yW1[:, 128:256].T + yb1,
        ],
        axis=1,
    ).astype(f)  # [B, 512]
    CF = np.ascontiguousarray(
        np.broadcast_to(np.concatenate([b_hh[256:384], tb1]).astype(f), (BC, 256))
    )
    WhE = np.ascontiguousarray(
        np.hstack([W_hh[0:128].T, W_hh[128:256].T, yW1[:, 0:128].T]).astype(f)
    )
    WhF = np.ascontiguousarray(np.hstack([W_hh[256:384].T, tW1.T]).astype(f))

    Rvar = np.zeros((KVAR, 512), f)
    Rvar[:, 0:128] = W_ih[0:128, 128:177].T
    Rvar[:, 128:256] = W_ih[128:256, 128:177].T
    Rvar[:, 256:384] = W_ih[256:384, 128:177].T
    Rvar[0:48, 384:512] = yW1[:, 256:304].T
    ident = np.eye(128, dtype=f)

    # catvar [B, S, 49] = [a_emb[a] | t_emb[t] | y]
    catvar = np.empty((B, S, KVAR), f)
    catvar[:, :, 0:32] = a_emb[a]
    catvar[:, :, 32:48] = t_emb[t]
    catvar[:, :, 48] = y

    nc = _build_program()

    in_maps = []
    for c in range(NCORES):
        sl = slice(c * BC, (c + 1) * BC)
        cv = catvar[sl]  # [128, S, 49]
        # -> [NG, KVAR, SG*BC]: catT4[g, k, j*BC + b] = cv[b, 4g+j, k]
        cT = np.ascontiguousarray(cv.transpose(1, 2, 0))  # [S, 49, 128]
        cT4 = np.ascontiguousarray(
            cT.reshape(NG, SG, KVAR, BC).transpose(0, 2, 1, 3)
        ).reshape(NG, KVAR, SG * BC)
        h16 = np.float16
        in_maps.append(
            {
                "h0T": np.ascontiguousarray(h0[sl].T).astype(h16),
                "h0": np.ascontiguousarray(h0[sl]).astype(h16),
                "catT4": cT4.astype(h16),
                "CE": np.ascontiguousarray(CE[sl]).astype(h16),
                "CF": CF.astype(h16),
                "WhE": WhE.astype(h16),
                "WhF": WhF.astype(h16),
                "Rvar": Rvar.astype(h16),
                "ident": ident.astype(h16),
            }
        )

    res = run_bass_kernel_spmd(nc, in_maps, list(range(NCORES)))

    y_seq = np.empty((B, S, 1), f)
    t_seq = np.empty((B, S, 4), f)
    h_seq = np.empty((B, S, 128), f)
    for c in range(NCORES):
        sl = slice(c * BC, (c + 1) * BC)
        OH = res.results[c]["OUTH"].astype(f)  # [128, S, 128]
        O2 = res.results[c]["OUT2"].astype(f)  # [128, S, 256]
        h_seq[sl] = OH
        h_seq[sl, 0, :] = h0[sl]
        # ty = 2*gelu(pre); heads: logits = gelu(pre) @ W.T + b
        t_seq[sl] = 0.5 * (O2[:, :, 0:128] @ tW2.T) + tb2
        y_seq[sl] = 0.5 * (O2[:, :, 128:256] @ yW2.T) + yb2

    return (y_seq, t_seq, h_seq)
